# revision 20
# baseline (speedup 1.0000x reference)
"""DendriticFullyConnected Trainium2 kernel — mixed bf16 / fp8-DoubleRow.

Math (per reference):
  x_c  = x[:, :409];  x_nc = x[:, 409:]
  state = sigmoid(x_nc @ W_non.T + b_non) - 1
  cluster = (x_c * coeff) @ W_nmda.T          # coeff = [1,2,...,2,1]
  pre = cluster + state
  out = pre^2 / (0.25 + pre^2)

Strategy: data-parallel over batch on 8 cores (1024 rows each), weights
replicated.  The contraction splits by precision sensitivity:

  nmda part (K=409->512, 4 k-tiles)  : bf16.  cluster hits the Hill directly
    (sigma~2, gain ~1), so fp8 here costs ~5e-2 rel err.  bf16 keeps it at
    ~3e-3 and runs at the same 1 cycle/row as fp32r, with half the DMA.
  non part (K=3687+bias->3840, 15 pairs of k-tiles): fp8 e4m3 with
    perf_mode=DoubleRow (2 fp8 weights per PE cell -> 256-deep contraction
    per matmul at ~0.55 cycle/row).  The sigmoid's <=0.25 gain squashes the
    fp8 quantization noise (measured 6.3e-3 rel-l2 end to end, vs 2e-2 gate).
    W_non/b_non are pre-scaled by 64 so sigma~1 lands mid e4m3 range (away
    from subnormals); the 1/64 is folded into the sigmoid activation scale.

Layouts are all host-prepared so every DMA is a straight contiguous copy:
  xnm [512, 1024/core] bf16;  xnn [15kp*128p, 2i*1024b] fp8 (i = DoubleRow
  half, logical k = kp*256 + i*128 + p);  wnm rows ot*128+p, cols kt*128+o;
  wnn rows ot*128+p, cols kp*256 + i*128 + o.  Bias rides as x-row 3687
  (ones) paired with b_non*64 in wnn.

Device: outT[o, b] = sum_k wt[k, o] xt[k, b] with W-stationary matmuls
(lhsT = w tile, rhs = cached x), two PSUM groups (nmda / non) per o-tile,
then sigmoid + Hill epilogue on ACT/DVE.  Host transposes back.

Scheduling: the x shard (~5 MB) is cached in SBUF.  Phase A runs the bf16
nmda phases of the first OT_AHEAD o-tiles (they only need the 1 MB xnm)
while the xnn fill streams (xnm + odd kp on GpSimd/SWDGE, even kp lazily
interleaved with the W stream on Sync/HWDGE).  Phases B/C are the plain
o-outer loop; W (622 KB/o-tile) streams on Sync, output stores ride the
ACT HWDGE queue.
"""

import numpy as np
import ml_dtypes

B = 8192
IN_F = 4096
OUT_F = 4096
IC = 409                      # clustering synapses
INC = IN_F - IC               # 3687
KD = 0.25                     # Hill k_d = k_a^n = 0.5^2
NCORES = 8
BLOC = B // NCORES            # 1024
OT = OUT_F // 128             # 32 output-row tiles
NBH = BLOC // 512             # 2 batch halves (512 = max fp32 matmul free dim)
OT_AHEAD = 10                 # o-tiles whose nmda phase covers the x fill

KNM_PAD = 512                 # nmda contraction, padded (4 k-tiles, bf16)
KNM_TILES = 4
KNN = INC + 1                 # 3688: non contraction + bias row
KP = 15                       # fp8 DoubleRow k-pairs (15 * 256 = 3840)
KNN_PAD = KP * 256
S_W = 64.0                    # fp8 pre-scale on W_non/b_non

_nc_cache = []


def _build():
    import concourse.bacc as bacc
    import concourse.tile as tile
    import concourse.mybir as mybir

    f32 = mybir.dt.float32
    bf16 = mybir.dt.bfloat16
    f8 = mybir.dt.float8e4
    ACT = mybir.ActivationFunctionType
    DR = mybir.MatmulPerfMode.DoubleRow

    nc = bacc.Bacc(None, target_bir_lowering=False)
    xnm = nc.dram_tensor("xnm", [KNM_PAD, BLOC], bf16, kind="ExternalInput")
    xnn = nc.dram_tensor("xnn", [KP * 128, 2 * BLOC], f8, kind="ExternalInput")
    wnm = nc.dram_tensor("wnm", [OUT_F, KNM_PAD], bf16, kind="ExternalInput")
    wnn = nc.dram_tensor("wnn", [OUT_F, KP * 256], f8, kind="ExternalInput")
    outT = nc.dram_tensor("outT", [OUT_F, BLOC], bf16, kind="ExternalOutput")

    with tile.TileContext(nc) as tc:
        with (
            tc.tile_pool(name="xpool", bufs=1) as xpool,
            tc.tile_pool(name="wmpool", bufs=6) as wmpool,
            tc.tile_pool(name="wnpool", bufs=4) as wnpool,
            tc.tile_pool(name="nmpool", bufs=24) as nmpool,
            tc.tile_pool(name="tmp", bufs=8) as tmp,
            tc.tile_pool(name="opool", bufs=8) as opool,
            tc.tile_pool(name="psum", bufs=8, space="PSUM") as psum,
        ):
            # ── x cache fill ────────────────────────────────────────────
            # xm k-tiles are split in batch halves across the ACT and GpSimd
            # queues so each tile lands ~1.7 us after the previous one —
            # pacing the phase-A k-outer sweep (1.7 us of PE work per tile).
            # xnn: odd kp on GpSimd, kp 10/12/14 on ACT, the rest lazily
            # interleaved with the W stream on Sync (which stays W-first).
            xm = []
            for kt in range(KNM_TILES):
                t = xpool.tile([128, BLOC], bf16, tag=f"xm{kt}")
                src = xnm[kt * 128 : (kt + 1) * 128, :]
                nc.scalar.dma_start(t[:, 0:512], src[:, 0:512])
                nc.gpsimd.dma_start(t[:, 512:], src[:, 512:])
                xm.append(t)
            xn = []
            x_pending = []
            for kp in range(KP):
                t = xpool.tile([128, 2, BLOC], f8, tag=f"xn{kp}")
                src = xnn[kp * 128 : (kp + 1) * 128, :].rearrange(
                    "p (i b) -> p i b", i=2
                )
                if kp % 2 == 1:
                    nc.gpsimd.dma_start(t[:], src)
                elif kp >= 10:
                    nc.scalar.dma_start(t[:], src)
                else:
                    x_pending.append((t, src))
                xn.append(t)
            x_pending.reverse()  # pop() from the front of the schedule

            def feed_x(n):
                for _ in range(n):
                    if x_pending:
                        t, src = x_pending.pop()
                        nc.sync.dma_start(t[:], src)

            def osl(ot):
                return slice(ot * 128, (ot + 1) * 128)

            def bsl(bh):
                return slice(bh * 512, (bh + 1) * 512)

            def load_wm(ot):
                wg = wmpool.tile([128, KNM_TILES, 128], bf16, tag="wm", name=f"wm_{ot}")
                nc.sync.dma_start(
                    wg[:], wnm[osl(ot), :].rearrange("p (k o) -> p k o", k=KNM_TILES)
                )
                feed_x(1)
                return wg

            def nmda_group(ots):
                # k-OUTER over a group of o-tiles (<=4: psum budget): during
                # the x fill each arriving xm[kt] unlocks len(ots)*2 matmuls
                # instead of 2, keeping the PE fed while xnm streams in.
                wgs = [load_wm(ot) for ot in ots]
                psn = [
                    [
                        psum.tile([128, 512], f32, tag="ps", name=f"psn_{ot}_{i}")
                        for i in range(NBH)
                    ]
                    for ot in ots
                ]
                for kt in range(KNM_TILES):
                    for j in range(len(ots)):
                        for bh in range(NBH):
                            nc.tensor.matmul(
                                psn[j][bh][:],
                                lhsT=wgs[j][:, kt, :],
                                rhs=xm[kt][:, bsl(bh)],
                                start=(kt == 0),
                                stop=(kt == KNM_TILES - 1),
                            )
                nms = []
                for j, ot in enumerate(ots):
                    nm = []
                    for bh in range(NBH):
                        t = nmpool.tile([128, 512], f32, tag="nm", name=f"nm_{ot}_{bh}")
                        nc.scalar.copy(t[:], psn[j][bh][:])
                        nm.append(t)
                    nms.append(nm)
                return nms

            def nmda_phase(ot):
                return nmda_group([ot])[0]

            wn_tiles = {}

            def prefetch_wn(ot):
                if ot not in wn_tiles:
                    wg = wnpool.tile([128, KP, 2, 128], f8, tag="wn", name=f"wn_{ot}")
                    nc.sync.dma_start(
                        wg[:],
                        wnn[osl(ot), :].rearrange("p (k i o) -> p k i o", k=KP, i=2),
                    )
                    feed_x(1)
                    wn_tiles[ot] = wg

            def get_wn(ot):
                prefetch_wn(ot)
                return wn_tiles.pop(ot)

            def non_phase(ot):
                wg = get_wn(ot)
                ps = [
                    psum.tile([128, 512], f32, tag="ps", name=f"ps_{ot}_{i}")
                    for i in range(NBH)
                ]
                for kp in range(KP):
                    for bh in range(NBH):
                        nc.tensor.matmul(
                            ps[bh][:],
                            lhsT=wg[:, kp, :, :],
                            rhs=xn[kp][:, :, bsl(bh)],
                            start=(kp == 0),
                            stop=(kp == KP - 1),
                            perf_mode=DR,
                        )
                return ps

            def epilogue_pair(ot, ps_pair, nm_pair):
                # psum = S_W*(z+b); pre = nm - sigmoid(-(z+b));
                # out = pre^2/(KD+pre^2) = 1 - KD/(KD+pre^2).  Chains
                # interleaved so ACT and DVE overlap across the batch halves.
                sig = [
                    tmp.tile([128, 512], f32, tag="t", name=f"sig_{ot}_{bh}")
                    for bh in range(NBH)
                ]
                rec = [
                    tmp.tile([128, 512], f32, tag="t", name=f"rec_{ot}_{bh}")
                    for bh in range(NBH)
                ]
                ob = [
                    opool.tile([128, 512], bf16, tag="o", name=f"ob_{ot}_{bh}")
                    for bh in range(NBH)
                ]
                for bh in range(NBH):
                    nc.scalar.activation(
                        sig[bh][:], ps_pair[bh][:], ACT.Sigmoid, scale=-1.0 / S_W
                    )
                for bh in range(NBH):
                    nc.vector.tensor_sub(sig[bh][:], nm_pair[bh][:], sig[bh][:])  # pre
                for bh in range(NBH):
                    nc.scalar.activation(nm_pair[bh][:], sig[bh][:], ACT.Square)
                for bh in range(NBH):
                    nc.vector.tensor_scalar_add(sig[bh][:], nm_pair[bh][:], KD)
                for bh in range(NBH):
                    nc.vector.reciprocal_approx_fast(rec[bh][:], sig[bh][:])
                for bh in range(NBH):
                    nc.vector.tensor_scalar(
                        ob[bh][:], rec[bh][:], -KD, 1.0,
                        mybir.AluOpType.mult, mybir.AluOpType.add,
                    )
                for bh in range(NBH):
                    # stores go on the GpSimd SWDGE queue, idle after the x
                    # fill: a store trigger blocked on data-readiness on the
                    # ACT queue would head-of-line-block the epilogue stream
                    # (costs ~25 us of end-of-kernel tail + psum-WAR stalls).
                    # The last o-tiles' stores go back on ACT/HWDGE: their
                    # data is ready when triggered (no HoL risk), and the
                    # final SWDGE drain (~7 us completion lag) leaves the
                    # teardown's critical path.
                    if ot >= OT - 2:
                        nc.scalar.dma_start(outT[osl(ot), bsl(bh)], ob[bh][:])
                    else:
                        nc.gpsimd.dma_start(outT[osl(ot), bsl(bh)], ob[bh][:])

            # ── Phase A: nmda for the first OT_AHEAD o-tiles (xnm only) ──
            nm_ahead = []
            groups = [
                list(range(g, min(g + 4, OT_AHEAD))) for g in range(0, OT_AHEAD, 4)
            ]
            for gi, grp in enumerate(groups):
                nm_ahead.extend(nmda_group(grp))
                if gi < 2:
                    prefetch_wn(gi)

            # ── Phase B: non + epilogue for the ahead o-tiles ──
            for ot in range(OT_AHEAD):
                ps = non_phase(ot)
                epilogue_pair(ot, ps, nm_ahead[ot])

            # ── Phase C: remaining o-tiles, plain o-outer loop ──
            for ot in range(OT_AHEAD, OT):
                nm = nmda_phase(ot)
                ps = non_phase(ot)
                epilogue_pair(ot, ps, nm)
    nc.compile()
    return nc


def _warmup():
    """Tiny throwaway NEFF run: the first execution after session start
    occasionally dies with NRT_EXEC_UNIT_UNRECOVERABLE; absorb that here."""
    import concourse.bacc as bacc
    import concourse.tile as tile
    import concourse.mybir as mybir
    from concourse.bass_utils import run_bass_kernel_spmd

    nc = bacc.Bacc(None, target_bir_lowering=False)
    a = nc.dram_tensor("a", [128, 128], mybir.dt.float32, kind="ExternalInput")
    b = nc.dram_tensor("b", [128, 128], mybir.dt.float32, kind="ExternalOutput")
    with tile.TileContext(nc) as tc:
        with tc.tile_pool(name="p", bufs=1) as pool:
            t = pool.tile([128, 128], mybir.dt.float32)
            nc.sync.dma_start(t[:], a[:])
            nc.sync.dma_start(b[:], t[:])
    nc.compile()
    ins = [{"a": np.zeros((128, 128), np.float32)} for _ in range(NCORES)]
    for _ in range(3):
        try:
            run_bass_kernel_spmd(nc, ins, core_ids=list(range(NCORES)))
            return
        except Exception:
            continue


def kernel(x, W_nmda, W_non, b_non):
    from concourse.bass_utils import run_bass_kernel_spmd

    x = np.asarray(x, dtype=np.float32)
    W_nmda = np.asarray(W_nmda, dtype=np.float32)
    W_non = np.asarray(W_non, dtype=np.float32)
    b_non = np.asarray(b_non, dtype=np.float32)

    coeff = np.full((IC,), 2.0, dtype=np.float32)
    coeff[0] = 1.0
    coeff[-1] = 1.0

    bf16 = ml_dtypes.bfloat16
    f8 = ml_dtypes.float8_e4m3

    # x, nmda part: [512, B] bf16
    xTm = np.zeros((KNM_PAD, B), dtype=np.float32)
    xTm[0:IC] = x[:, :IC].T
    xTm = xTm.astype(bf16)

    # x, non part: logical k = kp*256 + i*128 + p -> [15*128 rows, 2*B] fp8
    xTn = np.zeros((KNN_PAD, B), dtype=np.float32)
    xTn[0:INC] = x[:, IC:].T
    xTn[INC] = 1.0  # bias row
    xTn = (
        xTn.reshape(KP, 2, 128, B).transpose(0, 2, 1, 3).reshape(KP * 128, 2 * B)
    ).astype(f8)

    # W, nmda part: row ot*128+p, col kt*128+o, bf16
    wTm = np.zeros((KNM_PAD, OUT_F), dtype=np.float32)
    wTm[0:IC] = (W_nmda * coeff[None, :]).T
    wnm = (
        wTm.reshape(KNM_TILES, 128, OT, 128)
        .transpose(2, 1, 0, 3)
        .reshape(OUT_F, KNM_PAD)
    ).astype(bf16)

    # W, non part (pre-scaled by S_W): row ot*128+p, col kp*256+i*128+o, fp8
    wTn = np.zeros((KNN_PAD, OUT_F), dtype=np.float32)
    wTn[0:INC] = W_non.T * S_W
    wTn[INC] = b_non * S_W
    wnn = (
        wTn.reshape(KP, 2, 128, OT, 128)
        .transpose(3, 2, 0, 1, 4)
        .reshape(OUT_F, KP * 256)
    ).astype(f8)

    in_maps = [
        {
            "xnm": np.ascontiguousarray(
                xTm.reshape(KNM_PAD, NCORES, BLOC)[:, c, :]
            ),
            "xnn": np.ascontiguousarray(
                xTn.reshape(KP * 128, 2, NCORES, BLOC)[:, :, c, :].reshape(
                    KP * 128, 2 * BLOC
                )
            ),
            "wnm": wnm,
            "wnn": wnn,
        }
        for c in range(NCORES)
    ]

    if not _nc_cache:
        _warmup()
        _nc_cache.append(_build())
    nc = _nc_cache[0]

    res = None
    last_exc = None
    for _attempt in range(3):
        try:
            res = run_bass_kernel_spmd(nc, in_maps, core_ids=list(range(NCORES)))
            break
        except Exception as e:  # transient device errors (e.g. first-run NRT hiccup)
            last_exc = e
    if res is None:
        raise last_exc

    global LAST_RESULT
    LAST_RESULT = res

    out = np.empty((B, OUT_F), dtype=np.float32)
    for c in range(NCORES):
        out[c * BLOC : (c + 1) * BLOC] = res.results[c]["outT"].astype(np.float32).T
    return out


LAST_RESULT = None
